# revision 29
# baseline (speedup 1.0000x reference)
"""Distributed attention kernel for 8 TRN2 NeuronCores.

Problem: cross-attention (q from target, k/v from reference) with
B=2, N=M=2048, C=1024, H=16 heads, hd=64, followed by an output
projection with bias.

Sharding (data + head parallel):
  core c in 0..7 owns heads {2c, 2c+1} for BOTH batches. It computes
  K^T/Q^T/V for its heads and attention (softmax over keys), producing
  x_local^T [128ch, 2048m] per batch. The exchange is CHUNKED: per
  batch, TWO AllToAlls of [8, 128ch, 128m] each — chunk A covers
  m-tiles 0-1 (fired as soon as they are staged, overlapping the rest
  of attention), chunk B covers m-tiles 2-3. After the exchange core c
  owns output rows {c*128 + [0,128)} and {1024 + c*128 + [0,128)} of
  each batch with ALL 1024 channels and applies the full Wproj
  (replicated) + bias. Only the last chunk's collective + projection
  are exposed at the tail.

Softmax denominators come free as a ones-column appended to V. The
per-m-tile normalization is fully on-chip: the denominator row is
partition-broadcast on GpSimd, inverted with a fast-approximate
reciprocal on DVE, and multiplied into the staged x tile — no DRAM
bounces and no 4us 2-partition reciprocals. The projection bias-add
runs on DVE (tensor_scalar_add) so the Scalar engine does nothing but
the exp stream.

DMA issue cost (~0.7us of sequencer time per dma_start) dominated the
old startup, so bulk loads are merged: each weight is ONE dma_start,
activations stream in [128, 2, 2048] granules (one dma_start each)
spread across four engine queues, and each m-tile stages with ONE
dma_start. Matmuls run in bf16 (f32 PSUM accumulation).
"""

import functools

import numpy as np

B = 2
N = 2048  # reference rows (keys)
M = 2048  # target rows (queries)
C = 1024
H = 16
HD = 64
NCORES = 8
HPC = 2  # heads per core
CHPC = HPC * HD  # 128 channels per core
MBLK = 128  # output rows owned per core per (batch, chunk)
MT = 512  # attention m-tile
KC = N // 128  # 16 key chunks
CC = C // 128  # 8 contraction chunks
NMT = M // MT  # 4 m-tiles per batch
NG = CC // 2  # 4 activation granules per (tensor, batch)


@functools.lru_cache(maxsize=1)
def _build():
    import concourse.bacc as bacc
    import concourse.mybir as mybir
    import concourse.tile as tile

    fp32 = mybir.dt.float32
    bf16 = mybir.dt.bfloat16
    f8e4 = mybir.dt.float8e4
    AF = mybir.ActivationFunctionType

    nc = bacc.Bacc("TRN2", target_bir_lowering=False, debug=False, num_devices=NCORES)

    xrefT = nc.dram_tensor("xrefT", [B, C, N], bf16, kind="ExternalInput")
    xtgtT = nc.dram_tensor("xtgtT", [B, C, M], bf16, kind="ExternalInput")
    wq = nc.dram_tensor("wq", [C, CHPC], bf16, kind="ExternalInput")
    wk = nc.dram_tensor("wk", [C, CHPC], bf16, kind="ExternalInput")
    wv = nc.dram_tensor("wv", [C, CHPC], bf16, kind="ExternalInput")
    wproj = nc.dram_tensor("wproj", [C, C], bf16, kind="ExternalInput")
    bproj = nc.dram_tensor("bproj", [C], fp32, kind="ExternalInput")
    # out[b, f, :, :] = rows [1024*f + c*128, 1024*f + (c+1)*128) of batch b
    out = nc.dram_tensor("out", [B, 2, C, MBLK], fp32, kind="ExternalOutput")

    with tile.TileContext(nc) as tc:
        with (
            tc.tile_pool(name="wpool", bufs=1) as wpool,
            tc.tile_pool(name="xgpool", bufs=24) as xgpool,
            tc.tile_pool(name="kqv", bufs=1) as kqv,
            tc.tile_pool(name="epool", bufs=4) as epool,
            tc.tile_pool(name="xupool", bufs=3) as xupool,
            tc.tile_pool(name="stage", bufs=3) as stpool,
            tc.tile_pool(name="rpool", bufs=2) as rpool,
            tc.tile_pool(name="ppool", bufs=4) as ppool,
            tc.tile_pool(name="opool", bufs=2) as opool,
            tc.tile_pool(name="psA", bufs=3, space="PSUM") as psA,
            tc.tile_pool(name="psO", bufs=2, space="PSUM") as psO,
            tc.tile_pool(name="dram", bufs=1, space="DRAM") as dpool,
        ):
            # NOTE: no startup barrier collective — a collective trigger
            # blocks its sequencer until the CC engine is free, so an early
            # barrier that sits on the CC until peers launch would cascade
            # delays into every chunked AllToAll fire behind it.

            # ---- weight loads: ONE dma_start per weight, spread engines ----
            wk_sb = wpool.tile([128, CC, CHPC], bf16, name="wk_sb")
            wv_sb = wpool.tile([128, CC, CHPC], bf16, name="wv_sb")
            wq_sb = wpool.tile([128, CC, CHPC], bf16, name="wq_sb")
            nc.sync.dma_start(wk_sb[:], wk.ap().rearrange("(c p) m -> p c m", p=128))
            nc.scalar.dma_start(wv_sb[:], wv.ap().rearrange("(c p) m -> p c m", p=128))
            nc.gpsimd.dma_start(wq_sb[:], wq.ap().rearrange("(c p) m -> p c m", p=128))

            kT = [kqv.tile([128, N], bf16, tag=f"kT{b}", name=f"kT{b}") for b in range(B)]
            qT = [kqv.tile([128, M], bf16, tag=f"qT{b}", name=f"qT{b}") for b in range(B)]
            vA = [
                kqv.tile([128, KC, HPC, HD + 1], bf16, tag=f"vA{b}", name=f"vA{b}")
                for b in range(B)
            ]
            for b in range(B):
                nc.vector.memset(vA[b][:, :, :, HD:HD + 1], 1.0)

            # exchange buffers: [slot, 128ch, 128m] per (batch, chunk)
            a2a_in = [
                [
                    dpool.tile(
                        [NCORES, CHPC, MBLK], bf16,
                        tag=f"a2a_in{b}{f}", name=f"a2a_in{b}{f}",
                    )
                    for f in range(2)
                ]
                for b in range(B)
            ]
            a2a_out = [
                [
                    dpool.tile(
                        [NCORES, CHPC, MBLK], bf16,
                        tag=f"a2a_out{b}{f}", name=f"a2a_out{b}{f}",
                    )
                    for f in range(2)
                ]
                for b in range(B)
            ]

            # ---- activation chunk loads: 2D [128, N] per cc chunk.
            # NOTE: keep DMA access patterns 2D/contiguous-per-partition —
            # multi-segment patterns cost 5-19us of sequencer issue time
            # (DIRECT2D descriptor generation) vs ~0.7us for plain 2D.
            xg = {}  # (which, b, cc) -> sbuf chunk tile

            def load_granules(which, b):
                src = xrefT if which == "r" else xtgtT
                engs = [nc.sync, nc.scalar, nc.gpsimd]
                for cc in range(CC):
                    t = xgpool.tile([128, N], bf16, tag="xg", name=f"xg{which}{b}_{cc}")
                    xg[(which, b, cc)] = t
                    engs[cc % len(engs)].dma_start(
                        t[:], src[b, cc * 128:(cc + 1) * 128, :]
                    )

            def xap(which, b, cc):
                return xg[(which, b, cc)]

            load_granules("r", 0)
            load_granules("t", 0)

            def kt_half(b, half, w_sb, dstT, which):
                ps = psA.tile([128, 2 * MT], fp32, tag="big", name=f"ps{which}{b}{half}")
                for nt in range(2):
                    g = half * 2 + nt
                    for cc in range(CC):
                        nc.tensor.matmul(
                            ps[:, nt * MT:(nt + 1) * MT],
                            lhsT=w_sb[:, cc, :],
                            rhs=xap(which, b, cc)[:, g * MT:(g + 1) * MT],
                            start=(cc == 0),
                            stop=(cc == CC - 1),
                        )
                nc.vector.tensor_copy(
                    dstT[:, half * 2 * MT:(half + 1) * 2 * MT], ps[:]
                )

            def v_half(b, half):
                ps = psA.tile([128, 2 * MT], fp32, tag="big", name=f"psv{b}{half}")
                for k in range(8):
                    kc = half * 8 + k
                    for cc in range(CC):
                        nc.tensor.matmul(
                            ps[:, k * 128:(k + 1) * 128],
                            lhsT=xap("r", b, cc)[:, kc * 128:(kc + 1) * 128],
                            rhs=wv_sb[:, cc, :],
                            start=(cc == 0),
                            stop=(cc == CC - 1),
                        )
                nc.vector.tensor_copy(
                    vA[b][:, half * 8:(half + 1) * 8, :, 0:HD],
                    ps[:].rearrange("p (k h d) -> p k h d", k=8, h=HPC),
                )

            scale = float(HD) ** -0.5

            def attn_mt(b, mt):
                po = [
                    psO.tile([HD + 1, MT], fp32, tag="o", name=f"po{h}")
                    for h in range(HPC)
                ]

                def av_pair(kc, eS):
                    for h in range(HPC):
                        for j in range(2):
                            nc.tensor.matmul(
                                po[h][:],
                                lhsT=vA[b][:, kc + j, h, :],
                                rhs=eS[h][:, j, :],
                                start=(kc == 0 and j == 0),
                                stop=(kc == KC - 2 and j == 1),
                            )

                # software-pipelined by one kc-pair: the AV of pair k is
                # emitted AFTER the S^T of pair k+1, so the PE always has
                # wait-free work while the ACT engine streams exps, and exps
                # run back-to-back (ACT is the co-bottleneck engine).
                prev = None
                for kc in range(0, KC, 2):
                    pss = [
                        psA.tile([128, 2 * MT], fp32, tag="big", name="pss")
                        for _ in range(HPC)
                    ]
                    for j in range(2):
                        # the two heads sit at partitions 0-63 / 64-127
                        for h in range(HPC):
                            nc.tensor.matmul(
                                pss[h][:, j * MT:(j + 1) * MT],
                                lhsT=kT[b][h * HD:(h + 1) * HD, (kc + j) * 128:(kc + j + 1) * 128],
                                rhs=qT[b][h * HD:(h + 1) * HD, mt * MT:(mt + 1) * MT],
                                start=True,
                                stop=True,
                            )
                    eS = [
                        epool.tile([128, 2, MT], bf16, tag="eS", name="eS")
                        for _ in range(HPC)
                    ]
                    for h in range(HPC):
                        nc.scalar.activation(
                            eS[h][:].rearrange("p a b -> p (a b)"),
                            pss[h][:],
                            AF.Exp,
                            scale=scale,
                        )
                    if prev is not None:
                        av_pair(*prev)
                    prev = (kc, eS)
                av_pair(*prev)
                return po

            def norm_mt(b, mt, po):
                # on-chip normalize + stage: x rows copied off PSUM on GpSimd
                # (frees po fast), denominator row partition-broadcast on
                # GpSimd, fast-approx reciprocal + multiply on DVE, then ONE
                # staging dma_start into the chunk buffer.
                f, t = mt // 2, mt % 2
                pF = [
                    xupool.tile([HD, MT], fp32, tag=f"pF{h}", name=f"pF{b}{mt}{h}")
                    for h in range(HPC)
                ]
                dN = [
                    rpool.tile([1, MT], fp32, tag=f"dN{h}", name=f"dN{b}{mt}{h}")
                    for h in range(HPC)
                ]
                rbF = [
                    rpool.tile([HD, MT], fp32, tag=f"rbF{h}", name=f"rbF{b}{mt}{h}")
                    for h in range(HPC)
                ]
                rbI = [
                    rpool.tile([HD, MT], fp32, tag=f"rbI{h}", name=f"rbI{b}{mt}{h}")
                    for h in range(HPC)
                ]
                for h in range(HPC):
                    # PSUM -> SBUF (frees po fast; GpSimd cannot touch PSUM).
                    # The denominator row moves to partition 0 of its own tile
                    # (partition_broadcast reads the tile's partition 0).
                    nc.vector.tensor_copy(pF[h][:], po[h][0:HD, :])
                    nc.vector.tensor_copy(dN[h][:], po[h][HD:HD + 1, :])
                xst = stpool.tile([128, MT], bf16, tag="stage", name=f"xst{b}{mt}")
                for h in range(HPC):
                    nc.gpsimd.partition_broadcast(rbF[h][:], dN[h][:], channels=HD)
                for h in range(HPC):
                    nc.vector.reciprocal_approx_fast(rbI[h][:], rbF[h][:])
                    nc.vector.tensor_mul(
                        xst[h * HD:(h + 1) * HD, :],
                        pF[h][:],
                        rbI[h][:],
                    )
                nc.sync.dma_start(
                    a2a_in[b][f][4 * t:4 * t + 4].rearrange("s p m -> p s m"),
                    xst[:].rearrange("p (s m) -> p s m", s=4),
                )

            def fire_a2a(b, f):
                nc.gpsimd.collective_compute(
                    "AllToAll",
                    mybir.AluOpType.bypass,
                    replica_groups=[list(range(NCORES))],
                    ins=[a2a_in[b][f][:].opt()],
                    outs=[a2a_out[b][f][:].opt()],
                )

            y_sb = {}

            def yload(b, f):
                y = ppool.tile([128, NCORES, MBLK], bf16, tag="y", name=f"y{b}{f}")
                y_sb[(b, f)] = y
                nc.sync.dma_start(
                    y[:], a2a_out[b][f][:].rearrange("s p m -> p s m")
                )

            def proj(b, f):
                y = y_sb[(b, f)]
                osb = opool.tile([128, CC, MBLK], fp32, tag="osb", name=f"osb{b}{f}")
                for oc in range(CC):
                    psb = psA.tile([128, 2 * MT], fp32, tag="big", name="pp")
                    ps = psb[:, 0:MBLK]
                    for cc in range(CC):
                        nc.tensor.matmul(
                            ps[:],
                            lhsT=wp_sb[:, cc, oc * 128:(oc + 1) * 128],
                            rhs=y[:, cc, :],
                            start=(cc == 0),
                            stop=(cc == CC - 1),
                        )
                    nc.vector.tensor_scalar_add(
                        osb[:, oc, :], ps[:], bias_sb[:, oc:oc + 1]
                    )
                nc.sync.dma_start(
                    out[b, f].rearrange("(c p) m -> p c m", p=128), osb[:]
                )

            # ================= emission schedule =================
            kt_half(0, 0, wk_sb, kT[0], "r")
            v_half(0, 0)
            kt_half(0, 1, wk_sb, kT[0], "r")
            v_half(0, 1)
            kt_half(0, 0, wq_sb, qT[0], "t")
            kt_half(0, 1, wq_sb, qT[0], "t")

            # batch-1 granule loads queued early (DMA runs ahead of compute)
            load_granules("r", 1)
            load_granules("t", 1)

            # wproj/bias on the scalar ring: transfers queue behind the b1
            # granules there and land mid-attention, well before proj needs them
            wp_sb = wpool.tile([128, CC, C], bf16, name="wp_sb")
            for cc in range(CC):
                nc.scalar.dma_start(
                    wp_sb[:, cc, :], wproj[cc * 128:(cc + 1) * 128, :]
                )
            bias_sb = wpool.tile([128, CC], fp32, name="bias_sb")
            nc.scalar.dma_start(bias_sb[:], bproj.ap().rearrange("(a p) -> p a", p=128))

            # batch-1 QKV runs BEFORE attention: during the b1 granule DMA
            # the PE is idle anyway, and during attention the PE has zero
            # slack over the ACT exp stream — interleaving b1 QKV there
            # opened ~26us of exp-stream gaps (which also dropped the ACT
            # clock). Order: r-dependent pieces first (r1 arrives first).
            po = attn_mt(0, 0)
            norm_mt(0, 0, po)
            po = attn_mt(0, 1)
            norm_mt(0, 1, po)
            fire_a2a(0, 0)
            po = attn_mt(0, 2)
            norm_mt(0, 2, po)
            po = attn_mt(0, 3)
            norm_mt(0, 3, po)
            fire_a2a(0, 1)
            # ALL of batch-1 QKV at the batch boundary: its inputs (r1/t1) are
            # long since landed, so batch-0 attention starts right after
            # batch-0's own QKV instead of waiting for b1's projections
            kt_half(1, 0, wk_sb, kT[1], "r")
            v_half(1, 0)
            kt_half(1, 1, wk_sb, kT[1], "r")
            v_half(1, 1)
            kt_half(1, 0, wq_sb, qT[1], "t")
            kt_half(1, 1, wq_sb, qT[1], "t")
            po = attn_mt(1, 0)
            norm_mt(1, 0, po)
            po = attn_mt(1, 1)
            norm_mt(1, 1, po)
            fire_a2a(1, 0)
            po = attn_mt(1, 2)
            norm_mt(1, 2, po)
            po = attn_mt(1, 3)
            norm_mt(1, 3, po)
            fire_a2a(1, 1)
            yload(0, 0)
            yload(0, 1)
            yload(1, 0)
            yload(1, 1)
            proj(0, 0)
            proj(0, 1)
            proj(1, 0)
            proj(1, 1)

    nc.compile()
    return nc


def _shard_inputs(reference_data, target_data, Wq, Wkv, Wproj, bproj):
    import ml_dtypes

    bf16 = ml_dtypes.bfloat16
    xrefT = np.ascontiguousarray(
        np.asarray(reference_data, dtype=np.float32).transpose(0, 2, 1)
    ).astype(bf16)
    xtgtT = np.ascontiguousarray(
        np.asarray(target_data, dtype=np.float32).transpose(0, 2, 1)
    ).astype(bf16)
    Wq = np.asarray(Wq, dtype=np.float32)
    Wkv = np.asarray(Wkv, dtype=np.float32)
    Wproj_b = np.asarray(Wproj, dtype=np.float32).astype(bf16)
    bproj = np.asarray(bproj, dtype=np.float32)

    in_maps = []
    for c in range(NCORES):
        lo, hi = c * CHPC, (c + 1) * CHPC
        in_maps.append(
            {
                "xrefT": xrefT,
                "xtgtT": xtgtT,
                "wq": Wq[:, lo:hi].astype(bf16),
                "wk": Wkv[:, lo:hi].astype(bf16),
                "wv": Wkv[:, C + lo:C + hi].astype(bf16),
                "wproj": Wproj_b,
                "bproj": bproj,
            }
        )
    return in_maps


def _ensure_ntff_hook():
    """Register the axon NTFF profile hook if the image's antenv lacks it."""
    try:
        import antenv.axon_hooks  # noqa: F401

        return
    except ImportError:
        pass
    import sys
    import types

    import antenv

    mod = types.ModuleType("antenv.axon_hooks")
    state = {"hook": None}
    mod.set_axon_ntff_profile_hook = lambda h: state.__setitem__("hook", h)
    mod.get_axon_ntff_profile_hook = lambda: state["hook"]
    sys.modules["antenv.axon_hooks"] = mod
    antenv.axon_hooks = mod
    try:
        from trn_agent_boot.trn_boot import _ntff_profile_via_ctypes

        mod.set_axon_ntff_profile_hook(
            _ntff_profile_via_ctypes("/opt/axon/libaxon_pjrt.so")
        )
    except Exception:
        pass


def run(inputs: dict, trace: bool = False):
    """Compile (cached), run on 8 cores, return (full_output, BassKernelResults)."""
    from concourse.bass_utils import run_bass_kernel_spmd

    if trace:
        _ensure_ntff_hook()
    nc = _build()
    in_maps = _shard_inputs(**inputs)
    res = run_bass_kernel_spmd(
        nc, in_maps, core_ids=list(range(NCORES)), trace=trace
    )
    return _assemble(res), res


def _assemble(res):
    full = np.zeros((B, M, C), dtype=np.float32)
    for c in range(NCORES):
        blk = np.asarray(res.results[c]["out"], dtype=np.float32)  # [B, 2, C, 128]
        for b in range(B):
            for f in range(2):
                full[b, 1024 * f + c * MBLK:1024 * f + (c + 1) * MBLK, :] = (
                    blk[b, f].T
                )
    return full


def kernel(reference_data, target_data, Wq, Wkv, Wproj, bproj) -> np.ndarray:
    full, _ = run(
        {
            "reference_data": reference_data,
            "target_data": target_data,
            "Wq": Wq,
            "Wkv": Wkv,
            "Wproj": Wproj,
            "bproj": bproj,
        }
    )
    return full


# revision 30
# speedup vs baseline: 1.0665x; 1.0665x over previous
"""Distributed attention kernel for 8 TRN2 NeuronCores.

Problem: cross-attention (q from target, k/v from reference) with
B=2, N=M=2048, C=1024, H=16 heads, hd=64, followed by an output
projection with bias.

Sharding (data + head parallel):
  core c in 0..7 owns heads {2c, 2c+1} for BOTH batches. It computes
  K^T/Q^T/V for its heads and attention (softmax over keys), producing
  x_local^T [128ch, 2048m] per batch. The exchange is CHUNKED: per
  batch, TWO AllToAlls of [8, 128ch, 128m] each — chunk A covers
  m-tiles 0-1 (fired as soon as they are staged, overlapping the rest
  of attention), chunk B covers m-tiles 2-3. After the exchange core c
  owns output rows {c*128 + [0,128)} and {1024 + c*128 + [0,128)} of
  each batch with ALL 1024 channels and applies the full Wproj
  (replicated) + bias. Only the last chunk's collective + projection
  are exposed at the tail.

Softmax denominators come free as a ones-column appended to V. The
per-m-tile normalization is fully on-chip: the denominator row is
partition-broadcast on GpSimd, inverted with a fast-approximate
reciprocal on DVE, and multiplied into the staged x tile — no DRAM
bounces and no 4us 2-partition reciprocals. The projection bias-add
runs on DVE (tensor_scalar_add) so the Scalar engine does nothing but
the exp stream.

DMA issue cost (~0.7us of sequencer time per dma_start) dominated the
old startup, so bulk loads are merged: each weight is ONE dma_start,
activations stream in [128, 2, 2048] granules (one dma_start each)
spread across four engine queues, and each m-tile stages with ONE
dma_start. Matmuls run in bf16 (f32 PSUM accumulation).
"""

import functools

import numpy as np

B = 2
N = 2048  # reference rows (keys)
M = 2048  # target rows (queries)
C = 1024
H = 16
HD = 64
NCORES = 8
HPC = 2  # heads per core
CHPC = HPC * HD  # 128 channels per core
MBLK = 128  # output rows owned per core per (batch, chunk)
MT = 512  # attention m-tile
KC = N // 128  # 16 key chunks
CC = C // 128  # 8 contraction chunks
NMT = M // MT  # 4 m-tiles per batch
NG = CC // 2  # 4 activation granules per (tensor, batch)


@functools.lru_cache(maxsize=1)
def _build():
    import concourse.bacc as bacc
    import concourse.mybir as mybir
    import concourse.tile as tile

    fp32 = mybir.dt.float32
    bf16 = mybir.dt.bfloat16
    f8e4 = mybir.dt.float8e4
    AF = mybir.ActivationFunctionType

    nc = bacc.Bacc("TRN2", target_bir_lowering=False, debug=False, num_devices=NCORES)

    xrefT = nc.dram_tensor("xrefT", [B, C, N], bf16, kind="ExternalInput")
    xtgtT = nc.dram_tensor("xtgtT", [B, C, M], bf16, kind="ExternalInput")
    wq = nc.dram_tensor("wq", [C, CHPC], bf16, kind="ExternalInput")
    wk = nc.dram_tensor("wk", [C, CHPC], bf16, kind="ExternalInput")
    wv = nc.dram_tensor("wv", [C, CHPC], bf16, kind="ExternalInput")
    wproj = nc.dram_tensor("wproj", [C, C], bf16, kind="ExternalInput")
    bproj = nc.dram_tensor("bproj", [C], fp32, kind="ExternalInput")
    # out[b, f, :, :] = rows [1024*f + c*128, 1024*f + (c+1)*128) of batch b
    out = nc.dram_tensor("out", [B, 2, C, MBLK], fp32, kind="ExternalOutput")

    with tile.TileContext(nc) as tc:
        with (
            tc.tile_pool(name="wpool", bufs=1) as wpool,
            tc.tile_pool(name="xgpool", bufs=24) as xgpool,
            tc.tile_pool(name="kqv", bufs=1) as kqv,
            tc.tile_pool(name="epool", bufs=4) as epool,
            tc.tile_pool(name="xupool", bufs=3) as xupool,
            tc.tile_pool(name="stage", bufs=3) as stpool,
            tc.tile_pool(name="rpool", bufs=2) as rpool,
            tc.tile_pool(name="ppool", bufs=4) as ppool,
            tc.tile_pool(name="opool", bufs=2) as opool,
            tc.tile_pool(name="psA", bufs=3, space="PSUM") as psA,
            tc.tile_pool(name="psO", bufs=2, space="PSUM") as psO,
            tc.tile_pool(name="dram", bufs=1, space="DRAM") as dpool,
        ):
            # NOTE: no startup barrier collective — a collective trigger
            # blocks its sequencer until the CC engine is free, so an early
            # barrier that sits on the CC until peers launch would cascade
            # delays into every chunked AllToAll fire behind it.

            # ---- weight loads: ONE dma_start per weight, spread engines ----
            wk_sb = wpool.tile([128, CC, CHPC], bf16, name="wk_sb")
            wv_sb = wpool.tile([128, CC, CHPC], bf16, name="wv_sb")
            wq_sb = wpool.tile([128, CC, CHPC], bf16, name="wq_sb")
            nc.sync.dma_start(wk_sb[:], wk.ap().rearrange("(c p) m -> p c m", p=128))
            nc.scalar.dma_start(wv_sb[:], wv.ap().rearrange("(c p) m -> p c m", p=128))
            nc.gpsimd.dma_start(wq_sb[:], wq.ap().rearrange("(c p) m -> p c m", p=128))

            kT = [kqv.tile([128, N], bf16, tag=f"kT{b}", name=f"kT{b}") for b in range(B)]
            qT = [kqv.tile([128, M], bf16, tag=f"qT{b}", name=f"qT{b}") for b in range(B)]
            vA = [
                kqv.tile([128, KC, HPC, HD + 1], bf16, tag=f"vA{b}", name=f"vA{b}")
                for b in range(B)
            ]
            for b in range(B):
                nc.vector.memset(vA[b][:, :, :, HD:HD + 1], 1.0)

            # exchange buffers: [slot, 128ch, 128m] per (batch, chunk)
            a2a_in = [
                [
                    dpool.tile(
                        [NCORES, CHPC, MBLK], bf16,
                        tag=f"a2a_in{b}{f}", name=f"a2a_in{b}{f}",
                    )
                    for f in range(2)
                ]
                for b in range(B)
            ]
            a2a_out = [
                [
                    dpool.tile(
                        [NCORES, CHPC, MBLK], bf16,
                        tag=f"a2a_out{b}{f}", name=f"a2a_out{b}{f}",
                    )
                    for f in range(2)
                ]
                for b in range(B)
            ]

            # ---- activation chunk loads: 2D [128, N] per cc chunk.
            # NOTE: keep DMA access patterns 2D/contiguous-per-partition —
            # multi-segment patterns cost 5-19us of sequencer issue time
            # (DIRECT2D descriptor generation) vs ~0.7us for plain 2D.
            xg = {}  # (which, b, cc) -> sbuf chunk tile

            def load_granules(which, b):
                src = xrefT if which == "r" else xtgtT
                engs = [nc.sync, nc.scalar, nc.gpsimd]
                for cc in range(CC):
                    t = xgpool.tile([128, N], bf16, tag="xg", name=f"xg{which}{b}_{cc}")
                    xg[(which, b, cc)] = t
                    engs[cc % len(engs)].dma_start(
                        t[:], src[b, cc * 128:(cc + 1) * 128, :]
                    )

            def xap(which, b, cc):
                return xg[(which, b, cc)]

            load_granules("r", 0)
            load_granules("t", 0)

            def kt_half(b, half, w_sb, dstT, which):
                ps = psA.tile([128, 2 * MT], fp32, tag="big", name=f"ps{which}{b}{half}")
                for nt in range(2):
                    g = half * 2 + nt
                    for cc in range(CC):
                        nc.tensor.matmul(
                            ps[:, nt * MT:(nt + 1) * MT],
                            lhsT=w_sb[:, cc, :],
                            rhs=xap(which, b, cc)[:, g * MT:(g + 1) * MT],
                            start=(cc == 0),
                            stop=(cc == CC - 1),
                        )
                nc.vector.tensor_copy(
                    dstT[:, half * 2 * MT:(half + 1) * 2 * MT], ps[:]
                )

            def v_half(b, half):
                ps = psA.tile([128, 2 * MT], fp32, tag="big", name=f"psv{b}{half}")
                for k in range(8):
                    kc = half * 8 + k
                    for cc in range(CC):
                        nc.tensor.matmul(
                            ps[:, k * 128:(k + 1) * 128],
                            lhsT=xap("r", b, cc)[:, kc * 128:(kc + 1) * 128],
                            rhs=wv_sb[:, cc, :],
                            start=(cc == 0),
                            stop=(cc == CC - 1),
                        )
                nc.vector.tensor_copy(
                    vA[b][:, half * 8:(half + 1) * 8, :, 0:HD],
                    ps[:].rearrange("p (k h d) -> p k h d", k=8, h=HPC),
                )

            scale = float(HD) ** -0.5

            def attn_mt(b, mt):
                po = [
                    psO.tile([HD + 1, MT], fp32, tag="o", name=f"po{h}")
                    for h in range(HPC)
                ]

                def av_pair(kc, eS):
                    for h in range(HPC):
                        for j in range(2):
                            nc.tensor.matmul(
                                po[h][:],
                                lhsT=vA[b][:, kc + j, h, :],
                                rhs=eS[h][:, j, :],
                                start=(kc == 0 and j == 0),
                                stop=(kc == KC - 2 and j == 1),
                            )

                # software-pipelined by one kc-pair: the AV of pair k is
                # emitted AFTER the S^T of pair k+1, so the PE always has
                # wait-free work while the ACT engine streams exps, and exps
                # run back-to-back (ACT is the co-bottleneck engine).
                prev = None
                for kc in range(0, KC, 2):
                    pss = [
                        psA.tile([128, 2 * MT], fp32, tag="big", name="pss")
                        for _ in range(HPC)
                    ]
                    for j in range(2):
                        # the two heads sit at partitions 0-63 / 64-127
                        for h in range(HPC):
                            nc.tensor.matmul(
                                pss[h][:, j * MT:(j + 1) * MT],
                                lhsT=kT[b][h * HD:(h + 1) * HD, (kc + j) * 128:(kc + j + 1) * 128],
                                rhs=qT[b][h * HD:(h + 1) * HD, mt * MT:(mt + 1) * MT],
                                start=True,
                                stop=True,
                            )
                    eS = [
                        epool.tile([128, 2, MT], bf16, tag="eS", name="eS")
                        for _ in range(HPC)
                    ]
                    for h in range(HPC):
                        nc.scalar.activation(
                            eS[h][:].rearrange("p a b -> p (a b)"),
                            pss[h][:],
                            AF.Exp,
                            scale=scale,
                        )
                    if prev is not None:
                        av_pair(*prev)
                    prev = (kc, eS)
                av_pair(*prev)
                return po

            def norm_mt(b, mt, po):
                # on-chip normalize + stage: x rows copied off PSUM on GpSimd
                # (frees po fast), denominator row partition-broadcast on
                # GpSimd, fast-approx reciprocal + multiply on DVE, then ONE
                # staging dma_start into the chunk buffer.
                f, t = mt // 2, mt % 2
                pF = [
                    xupool.tile([HD, MT], fp32, tag=f"pF{h}", name=f"pF{b}{mt}{h}")
                    for h in range(HPC)
                ]
                dN = [
                    rpool.tile([1, MT], fp32, tag=f"dN{h}", name=f"dN{b}{mt}{h}")
                    for h in range(HPC)
                ]
                rbF = [
                    rpool.tile([HD, MT], fp32, tag=f"rbF{h}", name=f"rbF{b}{mt}{h}")
                    for h in range(HPC)
                ]
                rbI = [
                    rpool.tile([HD, MT], fp32, tag=f"rbI{h}", name=f"rbI{b}{mt}{h}")
                    for h in range(HPC)
                ]
                for h in range(HPC):
                    # PSUM -> SBUF (frees po fast; GpSimd cannot touch PSUM).
                    # The denominator row moves to partition 0 of its own tile
                    # (partition_broadcast reads the tile's partition 0).
                    nc.vector.tensor_copy(pF[h][:], po[h][0:HD, :])
                    nc.vector.tensor_copy(dN[h][:], po[h][HD:HD + 1, :])
                xst = stpool.tile([128, MT], bf16, tag="stage", name=f"xst{b}{mt}")
                for h in range(HPC):
                    nc.gpsimd.partition_broadcast(rbF[h][:], dN[h][:], channels=HD)
                for h in range(HPC):
                    nc.vector.reciprocal_approx_fast(rbI[h][:], rbF[h][:])
                    nc.vector.tensor_mul(
                        xst[h * HD:(h + 1) * HD, :],
                        pF[h][:],
                        rbI[h][:],
                    )
                nc.sync.dma_start(
                    a2a_in[b][f][4 * t:4 * t + 4].rearrange("s p m -> p s m"),
                    xst[:].rearrange("p (s m) -> p s m", s=4),
                )

            def fire_a2a(b, f):
                nc.gpsimd.collective_compute(
                    "AllToAll",
                    mybir.AluOpType.bypass,
                    replica_groups=[list(range(NCORES))],
                    ins=[a2a_in[b][f][:].opt()],
                    outs=[a2a_out[b][f][:].opt()],
                )

            y_sb = {}

            def yload(b, f):
                y = ppool.tile([128, NCORES, MBLK], bf16, tag="y", name=f"y{b}{f}")
                y_sb[(b, f)] = y
                nc.sync.dma_start(
                    y[:], a2a_out[b][f][:].rearrange("s p m -> p s m")
                )

            def proj(b, f):
                y = y_sb[(b, f)]
                osb = opool.tile([128, CC, MBLK], fp32, tag="osb", name=f"osb{b}{f}")
                for oc in range(CC):
                    psb = psA.tile([128, 2 * MT], fp32, tag="big", name="pp")
                    ps = psb[:, 0:MBLK]
                    for cc in range(CC):
                        nc.tensor.matmul(
                            ps[:],
                            lhsT=wp_sb[:, cc, oc * 128:(oc + 1) * 128],
                            rhs=y[:, cc, :],
                            start=(cc == 0),
                            stop=(cc == CC - 1),
                        )
                    nc.vector.tensor_scalar_add(
                        osb[:, oc, :], ps[:], bias_sb[:, oc:oc + 1]
                    )
                nc.sync.dma_start(
                    out[b, f].rearrange("(c p) m -> p c m", p=128), osb[:]
                )

            # ================= emission schedule =================
            kt_half(0, 0, wk_sb, kT[0], "r")
            v_half(0, 0)
            kt_half(0, 1, wk_sb, kT[0], "r")
            v_half(0, 1)
            kt_half(0, 0, wq_sb, qT[0], "t")
            kt_half(0, 1, wq_sb, qT[0], "t")

            # batch-1 granule loads queued early (DMA runs ahead of compute)
            load_granules("r", 1)
            load_granules("t", 1)

            # wproj/bias on the scalar ring: transfers queue behind the b1
            # granules there and land mid-attention, well before proj needs them
            wp_sb = wpool.tile([128, CC, C], bf16, name="wp_sb")
            for cc in range(CC):
                nc.scalar.dma_start(
                    wp_sb[:, cc, :], wproj[cc * 128:(cc + 1) * 128, :]
                )
            bias_sb = wpool.tile([128, CC], fp32, name="bias_sb")
            nc.scalar.dma_start(bias_sb[:], bproj.ap().rearrange("(a p) -> p a", p=128))

            # batch-1 QKV runs BEFORE attention: during the b1 granule DMA
            # the PE is idle anyway, and during attention the PE has zero
            # slack over the ACT exp stream — interleaving b1 QKV there
            # opened ~26us of exp-stream gaps (which also dropped the ACT
            # clock). Order: r-dependent pieces first (r1 arrives first).
            kt_half(1, 0, wk_sb, kT[1], "r")
            v_half(1, 0)
            kt_half(1, 1, wk_sb, kT[1], "r")
            v_half(1, 1)

            po = attn_mt(0, 0)
            norm_mt(0, 0, po)
            po = attn_mt(0, 1)
            norm_mt(0, 1, po)
            fire_a2a(0, 0)
            po = attn_mt(0, 2)
            norm_mt(0, 2, po)
            po = attn_mt(0, 3)
            norm_mt(0, 3, po)
            fire_a2a(0, 1)
            # q projection of batch 1 sits at the batch boundary: t1 (the
            # last DMA to land) is ready by then, and the single ~9us ACT
            # gap here is short enough not to downclock the Scalar engine
            # (a ~25us gap measurably drops the exp clock for the rest of
            # the kernel)
            kt_half(1, 0, wq_sb, qT[1], "t")
            kt_half(1, 1, wq_sb, qT[1], "t")
            po = attn_mt(1, 0)
            norm_mt(1, 0, po)
            po = attn_mt(1, 1)
            norm_mt(1, 1, po)
            fire_a2a(1, 0)
            po = attn_mt(1, 2)
            norm_mt(1, 2, po)
            po = attn_mt(1, 3)
            norm_mt(1, 3, po)
            fire_a2a(1, 1)
            yload(0, 0)
            yload(0, 1)
            yload(1, 0)
            yload(1, 1)
            proj(0, 0)
            proj(0, 1)
            proj(1, 0)
            proj(1, 1)

    nc.compile()
    return nc


def _shard_inputs(reference_data, target_data, Wq, Wkv, Wproj, bproj):
    import ml_dtypes

    bf16 = ml_dtypes.bfloat16
    xrefT = np.ascontiguousarray(
        np.asarray(reference_data, dtype=np.float32).transpose(0, 2, 1)
    ).astype(bf16)
    xtgtT = np.ascontiguousarray(
        np.asarray(target_data, dtype=np.float32).transpose(0, 2, 1)
    ).astype(bf16)
    Wq = np.asarray(Wq, dtype=np.float32)
    Wkv = np.asarray(Wkv, dtype=np.float32)
    Wproj_b = np.asarray(Wproj, dtype=np.float32).astype(bf16)
    bproj = np.asarray(bproj, dtype=np.float32)

    in_maps = []
    for c in range(NCORES):
        lo, hi = c * CHPC, (c + 1) * CHPC
        in_maps.append(
            {
                "xrefT": xrefT,
                "xtgtT": xtgtT,
                "wq": Wq[:, lo:hi].astype(bf16),
                "wk": Wkv[:, lo:hi].astype(bf16),
                "wv": Wkv[:, C + lo:C + hi].astype(bf16),
                "wproj": Wproj_b,
                "bproj": bproj,
            }
        )
    return in_maps


def _ensure_ntff_hook():
    """Register the axon NTFF profile hook if the image's antenv lacks it."""
    try:
        import antenv.axon_hooks  # noqa: F401

        return
    except ImportError:
        pass
    import sys
    import types

    import antenv

    mod = types.ModuleType("antenv.axon_hooks")
    state = {"hook": None}
    mod.set_axon_ntff_profile_hook = lambda h: state.__setitem__("hook", h)
    mod.get_axon_ntff_profile_hook = lambda: state["hook"]
    sys.modules["antenv.axon_hooks"] = mod
    antenv.axon_hooks = mod
    try:
        from trn_agent_boot.trn_boot import _ntff_profile_via_ctypes

        mod.set_axon_ntff_profile_hook(
            _ntff_profile_via_ctypes("/opt/axon/libaxon_pjrt.so")
        )
    except Exception:
        pass


def run(inputs: dict, trace: bool = False):
    """Compile (cached), run on 8 cores, return (full_output, BassKernelResults)."""
    from concourse.bass_utils import run_bass_kernel_spmd

    if trace:
        _ensure_ntff_hook()
    nc = _build()
    in_maps = _shard_inputs(**inputs)
    res = run_bass_kernel_spmd(
        nc, in_maps, core_ids=list(range(NCORES)), trace=trace
    )
    return _assemble(res), res


def _assemble(res):
    full = np.zeros((B, M, C), dtype=np.float32)
    for c in range(NCORES):
        blk = np.asarray(res.results[c]["out"], dtype=np.float32)  # [B, 2, C, 128]
        for b in range(B):
            for f in range(2):
                full[b, 1024 * f + c * MBLK:1024 * f + (c + 1) * MBLK, :] = (
                    blk[b, f].T
                )
    return full


def kernel(reference_data, target_data, Wq, Wkv, Wproj, bproj) -> np.ndarray:
    full, _ = run(
        {
            "reference_data": reference_data,
            "target_data": target_data,
            "Wq": Wq,
            "Wkv": Wkv,
            "Wproj": Wproj,
            "bproj": bproj,
        }
    )
    return full


# revision 31
# speedup vs baseline: 1.1224x; 1.0525x over previous
"""Distributed attention kernel for 8 TRN2 NeuronCores.

Problem: cross-attention (q from target, k/v from reference) with
B=2, N=M=2048, C=1024, H=16 heads, hd=64, followed by an output
projection with bias.

Sharding (data + head parallel):
  core c in 0..7 owns heads {2c, 2c+1} for BOTH batches. It computes
  K^T/Q^T/V for its heads and attention (softmax over keys), producing
  x_local^T [128ch, 2048m] per batch. The exchange is CHUNKED: per
  batch, TWO AllToAlls of [8, 128ch, 128m] each — chunk A covers
  m-tiles 0-1 (fired as soon as they are staged, overlapping the rest
  of attention), chunk B covers m-tiles 2-3. After the exchange core c
  owns output rows {c*128 + [0,128)} and {1024 + c*128 + [0,128)} of
  each batch with ALL 1024 channels and applies the full Wproj
  (replicated) + bias. Only the last chunk's collective + projection
  are exposed at the tail.

Softmax denominators come free as a ones-column appended to V. The
per-m-tile normalization is fully on-chip: the denominator row is
partition-broadcast on GpSimd, inverted with a fast-approximate
reciprocal on DVE, and multiplied into the staged x tile — no DRAM
bounces and no 4us 2-partition reciprocals. The projection bias-add
runs on DVE (tensor_scalar_add) so the Scalar engine does nothing but
the exp stream.

Scheduling notes (all measured on hardware):
- DMA access patterns stay 2D/contiguous-per-partition: a 3D pattern
  costs 5-19us of DIRECT2D issue time on the HWDGE sequencers vs
  ~0.7us for plain 2D. Activations load as [128, 2048] cc-chunks
  spread over the sync/scalar/gpsimd rings; batch-1 chunks are issued
  dependency-free (a dma trigger that waits on a tile-pool slot stalls
  its whole queue, including the exp stream behind it).
- A collective trigger blocks its sequencer until the CC engine
  accepts it, so nothing latency-critical (PSUM-freeing copies) may
  sit behind a fire on the gpsimd queue; po is released by DVE-only
  copies.
- Batch-1 K/V projections run before attention (the PE has no slack
  over the exp stream once attention starts); the q projection sits at
  the b0/b1 batch boundary, where its t1 input has landed and the
  single ~9us Scalar-engine gap is short enough not to downclock it
  (a ~25us gap drops the exp clock ~20% for the rest of the kernel).
- No startup barrier collective: it occupies the CC engine until the
  slowest-launched core arrives and delays every chunked AllToAll
  behind it.
Matmuls run in bf16 (f32 PSUM accumulation); fp8 is unusable here:
softmax-weight noise multiplies independent v values, so fp8 q/k
quantization (~3.6% rms) passes through to the output at full
strength, blowing the 2e-2 budget.
"""

import functools

import numpy as np

B = 2
N = 2048  # reference rows (keys)
M = 2048  # target rows (queries)
C = 1024
H = 16
HD = 64
NCORES = 8
HPC = 2  # heads per core
CHPC = HPC * HD  # 128 channels per core
MBLK = 128  # output rows owned per core per (batch, chunk)
MT = 512  # attention m-tile
KC = N // 128  # 16 key chunks
CC = C // 128  # 8 contraction chunks
NMT = M // MT  # 4 m-tiles per batch
NG = CC // 2  # 4 activation granules per (tensor, batch)


@functools.lru_cache(maxsize=1)
def _build():
    import concourse.bacc as bacc
    import concourse.mybir as mybir
    import concourse.tile as tile

    fp32 = mybir.dt.float32
    bf16 = mybir.dt.bfloat16
    f8e4 = mybir.dt.float8e4
    AF = mybir.ActivationFunctionType

    nc = bacc.Bacc("TRN2", target_bir_lowering=False, debug=False, num_devices=NCORES)

    xrefT = nc.dram_tensor("xrefT", [B, C, N], bf16, kind="ExternalInput")
    xtgtT = nc.dram_tensor("xtgtT", [B, C, M], bf16, kind="ExternalInput")
    wq = nc.dram_tensor("wq", [C, CHPC], bf16, kind="ExternalInput")
    wk = nc.dram_tensor("wk", [C, CHPC], bf16, kind="ExternalInput")
    wv = nc.dram_tensor("wv", [C, CHPC], bf16, kind="ExternalInput")
    wproj = nc.dram_tensor("wproj", [C, C], bf16, kind="ExternalInput")
    bproj = nc.dram_tensor("bproj", [C], fp32, kind="ExternalInput")
    # out[b, f, :, :] = rows [1024*f + c*128, 1024*f + (c+1)*128) of batch b
    out = nc.dram_tensor("out", [B, 2, C, MBLK], fp32, kind="ExternalOutput")

    with tile.TileContext(nc) as tc:
        with (
            tc.tile_pool(name="wpool", bufs=1) as wpool,
            tc.tile_pool(name="xgpool", bufs=24) as xgpool,
            tc.tile_pool(name="kqv", bufs=1) as kqv,
            tc.tile_pool(name="epool", bufs=4) as epool,
            tc.tile_pool(name="xupool", bufs=3) as xupool,
            tc.tile_pool(name="stage", bufs=3) as stpool,
            tc.tile_pool(name="rpool", bufs=2) as rpool,
            tc.tile_pool(name="ppool", bufs=4) as ppool,
            tc.tile_pool(name="opool", bufs=2) as opool,
            tc.tile_pool(name="psA", bufs=3, space="PSUM") as psA,
            tc.tile_pool(name="psO", bufs=2, space="PSUM") as psO,
            tc.tile_pool(name="dram", bufs=1, space="DRAM") as dpool,
        ):
            # NOTE: no startup barrier collective — a collective trigger
            # blocks its sequencer until the CC engine is free, so an early
            # barrier that sits on the CC until peers launch would cascade
            # delays into every chunked AllToAll fire behind it.

            # ---- weight loads: ONE dma_start per weight, spread engines ----
            wk_sb = wpool.tile([128, CC, CHPC], bf16, name="wk_sb")
            wv_sb = wpool.tile([128, CC, CHPC], bf16, name="wv_sb")
            wq_sb = wpool.tile([128, CC, CHPC], bf16, name="wq_sb")
            nc.sync.dma_start(wk_sb[:], wk.ap().rearrange("(c p) m -> p c m", p=128))
            nc.scalar.dma_start(wv_sb[:], wv.ap().rearrange("(c p) m -> p c m", p=128))
            nc.gpsimd.dma_start(wq_sb[:], wq.ap().rearrange("(c p) m -> p c m", p=128))

            kT = [kqv.tile([128, N], bf16, tag=f"kT{b}", name=f"kT{b}") for b in range(B)]
            qT = [kqv.tile([128, M], bf16, tag=f"qT{b}", name=f"qT{b}") for b in range(B)]
            vA = [
                kqv.tile([128, KC, HPC, HD + 1], bf16, tag=f"vA{b}", name=f"vA{b}")
                for b in range(B)
            ]
            for b in range(B):
                nc.vector.memset(vA[b][:, :, :, HD:HD + 1], 1.0)

            # exchange buffers: [slot, 128ch, 128m] per (batch, chunk)
            a2a_in = [
                [
                    dpool.tile(
                        [NCORES, CHPC, MBLK], bf16,
                        tag=f"a2a_in{b}{f}", name=f"a2a_in{b}{f}",
                    )
                    for f in range(2)
                ]
                for b in range(B)
            ]
            a2a_out = [
                [
                    dpool.tile(
                        [NCORES, CHPC, MBLK], bf16,
                        tag=f"a2a_out{b}{f}", name=f"a2a_out{b}{f}",
                    )
                    for f in range(2)
                ]
                for b in range(B)
            ]

            # ---- activation chunk loads: 2D [128, N] per cc chunk.
            # NOTE: keep DMA access patterns 2D/contiguous-per-partition —
            # multi-segment patterns cost 5-19us of sequencer issue time
            # (DIRECT2D descriptor generation) vs ~0.7us for plain 2D.
            xg = {}  # (which, b, cc) -> sbuf chunk tile

            def load_granules(which, b):
                src = xrefT if which == "r" else xtgtT
                engs = [nc.sync, nc.scalar, nc.gpsimd]
                for cc in range(CC):
                    t = xgpool.tile([128, N], bf16, tag="xg", name=f"xg{which}{b}_{cc}")
                    xg[(which, b, cc)] = t
                    engs[cc % len(engs)].dma_start(
                        t[:], src[b, cc * 128:(cc + 1) * 128, :]
                    )

            def xap(which, b, cc):
                return xg[(which, b, cc)]

            load_granules("r", 0)
            load_granules("t", 0)

            def kt_half(b, half, w_sb, dstT, which):
                ps = psA.tile([128, 2 * MT], fp32, tag="big", name=f"ps{which}{b}{half}")
                for nt in range(2):
                    g = half * 2 + nt
                    for cc in range(CC):
                        nc.tensor.matmul(
                            ps[:, nt * MT:(nt + 1) * MT],
                            lhsT=w_sb[:, cc, :],
                            rhs=xap(which, b, cc)[:, g * MT:(g + 1) * MT],
                            start=(cc == 0),
                            stop=(cc == CC - 1),
                        )
                nc.vector.tensor_copy(
                    dstT[:, half * 2 * MT:(half + 1) * 2 * MT], ps[:]
                )

            def v_half(b, half):
                ps = psA.tile([128, 2 * MT], fp32, tag="big", name=f"psv{b}{half}")
                for k in range(8):
                    kc = half * 8 + k
                    for cc in range(CC):
                        nc.tensor.matmul(
                            ps[:, k * 128:(k + 1) * 128],
                            lhsT=xap("r", b, cc)[:, kc * 128:(kc + 1) * 128],
                            rhs=wv_sb[:, cc, :],
                            start=(cc == 0),
                            stop=(cc == CC - 1),
                        )
                nc.vector.tensor_copy(
                    vA[b][:, half * 8:(half + 1) * 8, :, 0:HD],
                    ps[:].rearrange("p (k h d) -> p k h d", k=8, h=HPC),
                )

            scale = float(HD) ** -0.5

            def attn_mt(b, mt):
                po = [
                    psO.tile([HD + 1, MT], fp32, tag="o", name=f"po{h}")
                    for h in range(HPC)
                ]

                def av_pair(kc, eS):
                    for h in range(HPC):
                        for j in range(2):
                            nc.tensor.matmul(
                                po[h][:],
                                lhsT=vA[b][:, kc + j, h, :],
                                rhs=eS[h][:, j, :],
                                start=(kc == 0 and j == 0),
                                stop=(kc == KC - 2 and j == 1),
                            )

                # software-pipelined by one kc-pair: the AV of pair k is
                # emitted AFTER the S^T of pair k+1, so the PE always has
                # wait-free work while the ACT engine streams exps, and exps
                # run back-to-back (ACT is the co-bottleneck engine).
                prev = None
                for kc in range(0, KC, 2):
                    pss = [
                        psA.tile([128, 2 * MT], fp32, tag="big", name="pss")
                        for _ in range(HPC)
                    ]
                    for j in range(2):
                        # the two heads sit at partitions 0-63 / 64-127
                        for h in range(HPC):
                            nc.tensor.matmul(
                                pss[h][:, j * MT:(j + 1) * MT],
                                lhsT=kT[b][h * HD:(h + 1) * HD, (kc + j) * 128:(kc + j + 1) * 128],
                                rhs=qT[b][h * HD:(h + 1) * HD, mt * MT:(mt + 1) * MT],
                                start=True,
                                stop=True,
                            )
                    eS = [
                        epool.tile([128, 2, MT], bf16, tag="eS", name="eS")
                        for _ in range(HPC)
                    ]
                    for h in range(HPC):
                        nc.scalar.activation(
                            eS[h][:].rearrange("p a b -> p (a b)"),
                            pss[h][:],
                            AF.Exp,
                            scale=scale,
                        )
                    if prev is not None:
                        av_pair(*prev)
                    prev = (kc, eS)
                av_pair(*prev)
                return po

            def norm_mt(b, mt, po):
                # on-chip normalize + stage: x rows copied off PSUM on GpSimd
                # (frees po fast), denominator row partition-broadcast on
                # GpSimd, fast-approx reciprocal + multiply on DVE, then ONE
                # staging dma_start into the chunk buffer.
                f, t = mt // 2, mt % 2
                pF = [
                    xupool.tile([HD, MT], fp32, tag=f"pF{h}", name=f"pF{b}{mt}{h}")
                    for h in range(HPC)
                ]
                dN = [
                    rpool.tile([1, MT], fp32, tag=f"dN{h}", name=f"dN{b}{mt}{h}")
                    for h in range(HPC)
                ]
                rbF = [
                    rpool.tile([HD, MT], fp32, tag=f"rbF{h}", name=f"rbF{b}{mt}{h}")
                    for h in range(HPC)
                ]
                rbI = [
                    rpool.tile([HD, MT], fp32, tag=f"rbI{h}", name=f"rbI{b}{mt}{h}")
                    for h in range(HPC)
                ]
                for h in range(HPC):
                    # PSUM -> SBUF (frees po fast; GpSimd cannot touch PSUM).
                    # The denominator row moves to partition 0 of its own tile
                    # (partition_broadcast reads the tile's partition 0).
                    nc.vector.tensor_copy(pF[h][:], po[h][0:HD, :])
                    nc.vector.tensor_copy(dN[h][:], po[h][HD:HD + 1, :])
                xst = stpool.tile([128, MT], bf16, tag="stage", name=f"xst{b}{mt}")
                for h in range(HPC):
                    nc.gpsimd.partition_broadcast(rbF[h][:], dN[h][:], channels=HD)
                for h in range(HPC):
                    nc.vector.reciprocal_approx_fast(rbI[h][:], rbF[h][:])
                    nc.vector.tensor_mul(
                        xst[h * HD:(h + 1) * HD, :],
                        pF[h][:],
                        rbI[h][:],
                    )
                nc.sync.dma_start(
                    a2a_in[b][f][4 * t:4 * t + 4].rearrange("s p m -> p s m"),
                    xst[:].rearrange("p (s m) -> p s m", s=4),
                )

            def fire_a2a(b, f):
                nc.gpsimd.collective_compute(
                    "AllToAll",
                    mybir.AluOpType.bypass,
                    replica_groups=[list(range(NCORES))],
                    ins=[a2a_in[b][f][:].opt()],
                    outs=[a2a_out[b][f][:].opt()],
                )

            y_sb = {}

            def yload(b, f):
                y = ppool.tile([128, NCORES, MBLK], bf16, tag="y", name=f"y{b}{f}")
                y_sb[(b, f)] = y
                nc.sync.dma_start(
                    y[:], a2a_out[b][f][:].rearrange("s p m -> p s m")
                )

            def proj(b, f):
                y = y_sb[(b, f)]
                osb = opool.tile([128, CC, MBLK], fp32, tag="osb", name=f"osb{b}{f}")
                for oc in range(CC):
                    psb = psA.tile([128, 2 * MT], fp32, tag="big", name="pp")
                    ps = psb[:, 0:MBLK]
                    for cc in range(CC):
                        nc.tensor.matmul(
                            ps[:],
                            lhsT=wp_sb[:, cc, oc * 128:(oc + 1) * 128],
                            rhs=y[:, cc, :],
                            start=(cc == 0),
                            stop=(cc == CC - 1),
                        )
                    nc.vector.tensor_scalar_add(
                        osb[:, oc, :], ps[:], bias_sb[:, oc:oc + 1]
                    )
                nc.sync.dma_start(
                    out[b, f].rearrange("(c p) m -> p c m", p=128), osb[:]
                )

            # ================= emission schedule =================
            kt_half(0, 0, wk_sb, kT[0], "r")
            v_half(0, 0)
            kt_half(0, 1, wk_sb, kT[0], "r")
            v_half(0, 1)
            kt_half(0, 0, wq_sb, qT[0], "t")
            kt_half(0, 1, wq_sb, qT[0], "t")

            # batch-1 granule loads queued early (DMA runs ahead of compute)
            load_granules("r", 1)
            load_granules("t", 1)

            # wproj/bias on the scalar ring: transfers queue behind the b1
            # granules there and land mid-attention, well before proj needs them
            wp_sb = wpool.tile([128, CC, C], bf16, name="wp_sb")
            for cc in range(CC):
                nc.scalar.dma_start(
                    wp_sb[:, cc, :], wproj[cc * 128:(cc + 1) * 128, :]
                )
            bias_sb = wpool.tile([128, CC], fp32, name="bias_sb")
            nc.scalar.dma_start(bias_sb[:], bproj.ap().rearrange("(a p) -> p a", p=128))

            # batch-1 QKV runs BEFORE attention: during the b1 granule DMA
            # the PE is idle anyway, and during attention the PE has zero
            # slack over the ACT exp stream — interleaving b1 QKV there
            # opened ~26us of exp-stream gaps (which also dropped the ACT
            # clock). Order: r-dependent pieces first (r1 arrives first).
            kt_half(1, 0, wk_sb, kT[1], "r")
            v_half(1, 0)
            kt_half(1, 1, wk_sb, kT[1], "r")
            v_half(1, 1)

            po = attn_mt(0, 0)
            norm_mt(0, 0, po)
            po = attn_mt(0, 1)
            norm_mt(0, 1, po)
            fire_a2a(0, 0)
            po = attn_mt(0, 2)
            norm_mt(0, 2, po)
            po = attn_mt(0, 3)
            norm_mt(0, 3, po)
            fire_a2a(0, 1)
            # q projection of batch 1 sits at the batch boundary: t1 (the
            # last DMA to land) is ready by then, and the single ~9us ACT
            # gap here is short enough not to downclock the Scalar engine
            # (a ~25us gap measurably drops the exp clock for the rest of
            # the kernel)
            kt_half(1, 0, wq_sb, qT[1], "t")
            kt_half(1, 1, wq_sb, qT[1], "t")
            po = attn_mt(1, 0)
            norm_mt(1, 0, po)
            po = attn_mt(1, 1)
            norm_mt(1, 1, po)
            fire_a2a(1, 0)
            po = attn_mt(1, 2)
            norm_mt(1, 2, po)
            po = attn_mt(1, 3)
            norm_mt(1, 3, po)
            fire_a2a(1, 1)
            yload(0, 0)
            yload(0, 1)
            yload(1, 0)
            yload(1, 1)
            proj(0, 0)
            proj(0, 1)
            proj(1, 0)
            proj(1, 1)

    nc.compile()
    return nc


def _shard_inputs(reference_data, target_data, Wq, Wkv, Wproj, bproj):
    import ml_dtypes

    bf16 = ml_dtypes.bfloat16
    xrefT = np.ascontiguousarray(
        np.asarray(reference_data, dtype=np.float32).transpose(0, 2, 1)
    ).astype(bf16)
    xtgtT = np.ascontiguousarray(
        np.asarray(target_data, dtype=np.float32).transpose(0, 2, 1)
    ).astype(bf16)
    Wq = np.asarray(Wq, dtype=np.float32)
    Wkv = np.asarray(Wkv, dtype=np.float32)
    Wproj_b = np.asarray(Wproj, dtype=np.float32).astype(bf16)
    bproj = np.asarray(bproj, dtype=np.float32)

    in_maps = []
    for c in range(NCORES):
        lo, hi = c * CHPC, (c + 1) * CHPC
        in_maps.append(
            {
                "xrefT": xrefT,
                "xtgtT": xtgtT,
                "wq": Wq[:, lo:hi].astype(bf16),
                "wk": Wkv[:, lo:hi].astype(bf16),
                "wv": Wkv[:, C + lo:C + hi].astype(bf16),
                "wproj": Wproj_b,
                "bproj": bproj,
            }
        )
    return in_maps


def _ensure_ntff_hook():
    """Register the axon NTFF profile hook if the image's antenv lacks it."""
    try:
        import antenv.axon_hooks  # noqa: F401

        return
    except ImportError:
        pass
    import sys
    import types

    import antenv

    mod = types.ModuleType("antenv.axon_hooks")
    state = {"hook": None}
    mod.set_axon_ntff_profile_hook = lambda h: state.__setitem__("hook", h)
    mod.get_axon_ntff_profile_hook = lambda: state["hook"]
    sys.modules["antenv.axon_hooks"] = mod
    antenv.axon_hooks = mod
    try:
        from trn_agent_boot.trn_boot import _ntff_profile_via_ctypes

        mod.set_axon_ntff_profile_hook(
            _ntff_profile_via_ctypes("/opt/axon/libaxon_pjrt.so")
        )
    except Exception:
        pass


def run(inputs: dict, trace: bool = False):
    """Compile (cached), run on 8 cores, return (full_output, BassKernelResults)."""
    from concourse.bass_utils import run_bass_kernel_spmd

    if trace:
        _ensure_ntff_hook()
    nc = _build()
    in_maps = _shard_inputs(**inputs)
    res = run_bass_kernel_spmd(
        nc, in_maps, core_ids=list(range(NCORES)), trace=trace
    )
    return _assemble(res), res


def _assemble(res):
    full = np.zeros((B, M, C), dtype=np.float32)
    for c in range(NCORES):
        blk = np.asarray(res.results[c]["out"], dtype=np.float32)  # [B, 2, C, 128]
        for b in range(B):
            for f in range(2):
                full[b, 1024 * f + c * MBLK:1024 * f + (c + 1) * MBLK, :] = (
                    blk[b, f].T
                )
    return full


def kernel(reference_data, target_data, Wq, Wkv, Wproj, bproj) -> np.ndarray:
    full, _ = run(
        {
            "reference_data": reference_data,
            "target_data": target_data,
            "Wq": Wq,
            "Wkv": Wkv,
            "Wproj": Wproj,
            "bproj": bproj,
        }
    )
    return full
